# revision 1
# baseline (speedup 1.0000x reference)
"""Trainium2 Bass kernel for nn_CrossAggregator (gnn_message_passing).

out[g,o] = self[g]·W1[o,:] + ea_g^T A_o eb_g,  g=(b,m), A_o = W[o,128:].reshape(128,128)
ea/eb = masked means over 32 neighbors (t=0 / t=1).

Design (per core, batch/8 data-parallel, G=512 rows), all heavy data bf16:
- inputs packed on host into partition-major mega-tensors; DMA'd in 8-slab
  chunks (few, large DMAs: HWDGE generation was the f32 baseline's bottleneck).
- eb-side: masked-mean via K=128 bf16 matmuls with a banded selector (BIG) as
  stationary -> ebT [j,g] in PSUM; copied to SBUF as bf16.
- ea-side: masked-mean + partition-broadcast fused into bf16 matmuls
  (stationary = all-ones/32, row-group selected via tile_position) -> PSUM rep.
- mask multiplies on GPSIMD (tensor_tensor, stride-0 mask broadcast), bf16.
- outer-product chunks Pt on DVE: pt[j,(i,g)] = ebT[j,g] * rep_i[j,g], bf16 out.
- main contraction on PE: psum_out[o,g] += W2chunk_i^T @ pt_i (bf16, f32 acc).
- host does only layout transforms (shard/permute/pack/bf16 cast) + out transpose.
"""
import sys
import numpy as np

for _p in ("/opt/trn_rl_repo", "/root/.axon_site/_ro/trn_rl_repo"):
    if _p not in sys.path:
        sys.path.insert(0, _p)

B, M, TWO, NN, D = 1024, 4, 2, 32, 128
OUT = 128
NCORES = 8
BC = B // NCORES          # batches per core
G = BC * M                # 512 rows per core
NIG = D // 4              # 32 slabs of 4 j's (partition packing (q,n))
CH = 8                    # slabs per DMA chunk -> 4 chunks per side
NCHUNK = NIG // CH

_CACHE = {}


def _build_nc():
    import os
    import concourse.bacc as bacc_mod
    import concourse.mybir as mybir
    from concourse.tile import TileContext

    F32 = mybir.dt.float32
    BF16 = mybir.dt.bfloat16
    MUL = mybir.AluOpType.mult

    nc = bacc_mod.Bacc(None)

    d_naA = nc.declare_dram_parameter("naA", [128, NIG * G], BF16, isOutput=False)
    d_nbA = nc.declare_dram_parameter("nbA", [128, NIG * G], BF16, isOutput=False)
    d_maskA = nc.declare_dram_parameter("maskA", [128, G], BF16, isOutput=False)
    d_maskB = nc.declare_dram_parameter("maskB", [128, G], BF16, isOutput=False)
    d_selfT = nc.declare_dram_parameter("selfT", [D, G], BF16, isOutput=False)
    d_W1 = nc.declare_dram_parameter("W1a", [D, OUT], BF16, isOutput=False)
    d_W2 = nc.declare_dram_parameter("W2A", [D, NIG * 4 * OUT], BF16, isOutput=False)
    d_BIG = nc.declare_dram_parameter("BIG", [128, 252], BF16, isOutput=False)
    d_ones = nc.declare_dram_parameter("ones32", [128, 128], BF16, isOutput=False)
    d_out = nc.declare_dram_parameter("outT", [OUT, G], F32, isOutput=True)

    with TileContext(nc) as tc:
        with (
            tc.tile_pool(name="const", bufs=1) as cpool,
            tc.tile_pool(name="nb_raw", bufs=2) as nbpool,
            tc.tile_pool(name="nb_msk", bufs=2) as nbmpool,
            tc.tile_pool(name="na_raw", bufs=2) as napool,
            tc.tile_pool(name="na_msk", bufs=2) as nampool,
            tc.tile_pool(name="w2", bufs=2) as w2pool,
            tc.tile_pool(name="pt", bufs=3) as ptpool,
            tc.tile_pool(name="misc", bufs=1) as mpool,
            tc.tile_pool(name="ps_ebt", bufs=1, space="PSUM") as ps_ebt,
            tc.tile_pool(name="ps_rep", bufs=3, space="PSUM") as ps_rep,
            tc.tile_pool(name="ps_out", bufs=1, space="PSUM") as ps_out,
        ):
            # constants / small tensors (maskB first: it gates the first
            # eb mask multiply; everything else can trail the nb stream)
            maskB_t = cpool.tile([128, G], BF16, tag="mb")
            nc.sync.dma_start(out=maskB_t[:], in_=d_maskB[:])
            big_t = cpool.tile([128, 252], BF16, tag="big")
            nc.sync.dma_start(out=big_t[:], in_=d_BIG[:])
            maskA_t = cpool.tile([128, G], BF16, tag="ma")
            nc.sync.dma_start(out=maskA_t[:], in_=d_maskA[:])
            ones_t = cpool.tile([128, 128], BF16, tag="ones")
            nc.sync.dma_start(out=ones_t[:], in_=d_ones[:])
            selfT_t = cpool.tile([D, G], BF16, tag="sT")
            nc.sync.dma_start(out=selfT_t[:], in_=d_selfT[:])
            w1_t = cpool.tile([D, OUT], BF16, tag="w1")
            nc.sync.dma_start(out=w1_t[:], in_=d_W1[:])

            _loop_n = int(os.environ.get("KERNEL_LOOP", "0"))
            _amp = int(os.environ.get("KERNEL_AMP", "1"))
            from contextlib import nullcontext
            _ctx = tc.For_i(0, _loop_n, 1) if _loop_n else nullcontext()
            with _ctx:
              for _rep in range(_amp):
                # chunk emission: na DMA + DVE mask + W2 DMA for chunk c
                ma_tiles = {}
                w2_tiles = {}

                na_tiles = {}

                def emit_chunk_na_dma(c):
                    na_t = napool.tile([128, CH * G], BF16, tag="na")
                    nc.sync.dma_start(
                        out=na_t[:], in_=d_naA[:, c * CH * G : (c + 1) * CH * G]
                    )
                    na_tiles[c] = na_t

                def emit_chunk_mask(c):
                    na_t = na_tiles.pop(c)
                    ma_t = nampool.tile([128, CH * G], BF16, tag="ma8")
                    nc.vector.tensor_tensor(
                        out=ma_t[:].rearrange("p (s c) -> p s c", s=CH),
                        in0=na_t[:].rearrange("p (s c) -> p s c", s=CH),
                        in1=maskA_t[:][:, None, :].broadcast_to([128, CH, G]),
                        op=MUL,
                    )
                    ma_tiles[c] = ma_t

                def emit_chunk_w2(c):
                    w2_t = w2pool.tile([128, CH * 4 * OUT], BF16, tag="w2t")
                    nc.sync.dma_start(
                        out=w2_t[:],
                        in_=d_W2[:, c * CH * 4 * OUT : (c + 1) * CH * 4 * OUT],
                    )
                    w2_tiles[c] = w2_t

                def emit_chunk(c):
                    emit_chunk_na_dma(c)
                    emit_chunk_mask(c)
                    emit_chunk_w2(c)

                rep_tiles = {}

                def emit_rep(k):
                    ig = k // 2
                    s = ig % CH
                    ma_t = ma_tiles[ig // CH]
                    rep = ps_rep.tile([128, 2 * G], F32, tag="rep")
                    for u in range(2):
                        isub = 2 * (k % 2) + u
                        nc.tensor.matmul(
                            rep[:, G * u : G * (u + 1)],
                            ones_t[32 * isub : 32 * isub + 32, :],
                            ma_t[32 * isub : 32 * isub + 32, s * G : (s + 1) * G],
                            start=True,
                            stop=True,
                            tile_position=(32 * isub, 0),
                        )
                    rep_tiles[k] = rep

                # ---- EB phase: ebT[j, g] in PSUM via banded-selector matmuls ----
                # mask multiplies on DVE (bf16 all-SBUF -> 4x perf mode);
                # chunk-0 rep matmuls interleaved to fill PE while nb streams.
                p_ebt = ps_ebt.tile([128, G], F32, tag="ebt")
                nb_tiles = []
                for c in range(NCHUNK):
                    nb_t = nbpool.tile([128, CH * G], BF16, tag="nb")
                    nc.sync.dma_start(
                        out=nb_t[:], in_=d_nbA[:, c * CH * G : (c + 1) * CH * G]
                    )
                    if c == 0:
                        emit_chunk_na_dma(0)  # na0 DMA right behind nb0
                    mb_t = nbmpool.tile([128, CH * G], BF16, tag="mb8")
                    nc.vector.tensor_tensor(
                        out=mb_t[:].rearrange("p (s c) -> p s c", s=CH),
                        in0=nb_t[:].rearrange("p (s c) -> p s c", s=CH),
                        in1=maskB_t[:][:, None, :].broadcast_to([128, CH, G]),
                        op=MUL,
                    )
                    for u in range(CH):
                        jg = c * CH + u
                        nc.tensor.matmul(
                            p_ebt[:],
                            big_t[:, 124 - 4 * jg : 252 - 4 * jg],
                            mb_t[:, G * u : G * (u + 1)],
                            start=(jg == 0),
                            stop=(jg == NIG - 1),
                        )
                    if c == 1:
                        emit_chunk_mask(0)  # ma0 mask after mb0/mb1 on DVE
                    if c >= 2:
                        emit_rep(c - 2)  # rep k=0..1 while eb still streaming
                emit_chunk_w2(0)  # W2-0 DMA behind the whole nb stream
                ebT_sb = mpool.tile([128, G], BF16, tag="ebsb")
                nc.scalar.copy(out=ebT_sb[:], in_=p_ebt[:])

                # ---- MAIN phase ----
                p_out = ps_out.tile([OUT, G], F32, tag="out")
                nc.tensor.matmul(p_out[:], w1_t[:], selfT_t[:], start=True, stop=False)

                # pt scheduling: 'v' = DVE direct from PSUM (f32 in, 1x rate)
                #                'c' = Act copy to bf16 SBUF, Pool multiply
                #                'x' = Act copy to bf16 SBUF, DVE multiply (2x)
                ptsched = os.environ.get("PT_SCHED", "vcxvxcvxvcxvxcvx")
                pt_tiles = {}

                def emit_pt(k):
                    rep = rep_tiles.pop(k)
                    pt2 = ptpool.tile([128, 2 * G], BF16, tag="pt2")
                    mode = ptsched[k % len(ptsched)]
                    if mode == "v":
                        nc.vector.tensor_tensor(
                            out=pt2[:].rearrange("p (a c) -> p a c", a=2),
                            in0=ebT_sb[:][:, None, :].broadcast_to([128, 2, G]),
                            in1=rep[:].rearrange("p (a c) -> p a c", a=2),
                            op=MUL,
                        )
                    else:
                        rep_sb = ptpool.tile([128, 2 * G], BF16, tag="repsb")
                        nc.scalar.copy(out=rep_sb[:], in_=rep[:])
                        eng = nc.gpsimd if mode == "c" else nc.vector
                        eng.tensor_tensor(
                            out=pt2[:].rearrange("p (a c) -> p a c", a=2),
                            in0=ebT_sb[:][:, None, :].broadcast_to([128, 2, G]),
                            in1=rep_sb[:].rearrange("p (a c) -> p a c", a=2),
                            op=MUL,
                        )
                    pt_tiles[k] = pt2

                emit_pt(0)
                for k in range(64):  # pair k covers i = 2k, 2k+1 ; ig = k//2
                    ig = k // 2
                    c, s = ig // CH, ig % CH
                    if k % 16 == 6 and c + 1 < NCHUNK:
                        emit_chunk(c + 1)  # chunk-ahead prefetch
                    if k + 2 <= 63 and (k + 2) not in rep_tiles and (k + 2) not in pt_tiles:
                        emit_rep(k + 2)  # rep two steps ahead of main_k
                    if k + 1 <= 63 and (k + 1) not in pt_tiles:
                        emit_pt(k + 1)  # pt one step ahead of main_k
                    pt2 = pt_tiles.pop(k)
                    w2_t = w2_tiles[c]
                    for u in range(2):
                        isub = 2 * (k % 2) + u
                        nc.tensor.matmul(
                            p_out[:],
                            w2_t[:, s * 4 * OUT + isub * OUT : s * 4 * OUT + (isub + 1) * OUT],
                            pt2[:, G * u : G * (u + 1)],
                            start=False,
                            stop=(k == 63 and u == 1),
                        )

                out_sb = mpool.tile([OUT, G], F32, tag="osb")
                nc.scalar.copy(out=out_sb[:], in_=p_out[:])
                nc.sync.dma_start(out=d_out[:], in_=out_sb[:])

    nc.finalize()
    return nc


def _host_prep(self_vectors, neighbor_vectors, masks, W):
    import ml_dtypes

    f32 = np.float32
    bf16 = ml_dtypes.bfloat16
    sv = np.asarray(self_vectors, dtype=f32)
    nv = np.asarray(neighbor_vectors, dtype=f32)
    mk = np.asarray(masks, dtype=f32)
    Wf = np.asarray(W, dtype=f32)

    # per-core packs: partition p = (q, n) holds feature j = 4*ig + q
    # cols = (ig, g)
    nvc = nv.reshape(NCORES, G, TWO, NN, D)          # [c, g, t, n, d]

    def pack_side(t):
        arr = nvc[:, :, t]                            # [c, g, n, d]
        arr = arr.transpose(0, 3, 2, 1)               # [c, d, n, g]
        arr = arr.reshape(NCORES, NIG, 4, NN, G)      # [c, ig, q, n, g]
        arr = arr.transpose(0, 2, 3, 1, 4)            # [c, q, n, ig, g]
        return np.ascontiguousarray(
            arr.reshape(NCORES, 128, NIG * G).astype(bf16)
        )

    naA = pack_side(0)
    nbA = pack_side(1)

    mkc = mk.reshape(NCORES, G, TWO, NN)             # [c, g, t, n]
    mA = mkc[:, :, 0].transpose(0, 2, 1)             # [c, n, g]
    mB = mkc[:, :, 1].transpose(0, 2, 1)
    maskA = np.ascontiguousarray(
        np.broadcast_to(mA[:, None], (NCORES, 4, NN, G)).reshape(NCORES, 128, G).astype(bf16)
    )
    maskB = np.ascontiguousarray(
        np.broadcast_to(mB[:, None], (NCORES, 4, NN, G)).reshape(NCORES, 128, G).astype(bf16)
    )
    selfT = np.ascontiguousarray(
        sv.reshape(NCORES, G, D).transpose(0, 2, 1).astype(bf16)
    )  # [c, d, g]

    # shared weights
    W1a = np.ascontiguousarray(Wf[:, :D].T.astype(bf16))          # [d, o]
    w2 = Wf[:, D:].reshape(OUT, D, D)                             # [o, i, j]
    W2A = np.ascontiguousarray(
        w2.transpose(2, 1, 0)                                     # [j, i, o]
        .reshape(D, NIG, 4, OUT)                                  # [j, ig, isub, o]
        .reshape(D, NIG * 4 * OUT)
        .astype(bf16)
    )
    BIG = np.zeros((128, 252), f32)
    r = np.arange(128)
    BIG[r, 124 + r // 32] = 1.0 / 32.0
    BIG = BIG.astype(bf16)
    ones32 = np.full((128, 128), 1.0 / 32.0, bf16)

    in_maps = []
    for c in range(NCORES):
        in_maps.append(
            {
                "naA": naA[c],
                "nbA": nbA[c],
                "maskA": maskA[c],
                "maskB": maskB[c],
                "selfT": selfT[c],
                "W1a": W1a,
                "W2A": W2A,
                "BIG": BIG,
                "ones32": ones32,
            }
        )
    return in_maps


def kernel(self_vectors, neighbor_vectors, masks, W, b):
    from concourse.bass_utils import run_bass_kernel_spmd

    if "nc" not in _CACHE:
        _CACHE["nc"] = _build_nc()
    nc = _CACHE["nc"]
    in_maps = _host_prep(self_vectors, neighbor_vectors, masks, W)
    results = run_bass_kernel_spmd(nc, in_maps, list(range(NCORES))).results
    out = np.empty((B, M, OUT), np.float32)
    for c in range(NCORES):
        out[c * BC : (c + 1) * BC] = (
            results[c]["outT"].T.reshape(BC, M, OUT)
        )
    out += np.asarray(b, np.float32)[None, None, :]
    return out



# revision 12
# speedup vs baseline: 532.0512x; 532.0512x over previous
"""Trainium2 Bass kernel for nn_CrossAggregator (gnn_message_passing).

out[g,o] = self[g]·W1[o,:] + ea_g^T A_o eb_g,  g=(b,m), A_o = W[o,128:].reshape(128,128)
ea/eb = masked means over 32 neighbors (t=0 / t=1).

Design v3 (per core, batch/8 data-parallel, G=512 rows), all heavy data bf16:
- single DMA queue (sync), strict priority order: consts, nb, na, W2 —
  chunks land in-place in full-resident tiles (no pool-recycle stalls).
- eb-side: masked-mean via 32 bf16 matmuls with a banded selector (BIG) as
  stationary -> ebT [j,g] in PSUM; Act-copied to SBUF bf16; 3 partition
  rotations ebT[(p+32r)%128, g] via PE permutation matmuls + Act copies
  -> ebTall [128, 4G].
- ea-side: ONE matmul per slab with a block-diagonal ones/32 stationary (BD)
  broadcasts all 4 i's of the slab into 4 partition bands -> repQ [128,G]
  (32 rep passes instead of 128).
- pt: repQ PSUM -> Act copy to bf16 -> one DVE 2x multiply per slab against
  the 4 ebT rotations (in0 dense, in1 stride-0 broadcast) -> pth [128, 4G].
- main contraction: 4 matmuls per slab, stationary W2stat[ig,r][p,o] =
  W2[o, 4ig+p//32, (p+32r)%128]; the rotation is folded into host W2
  packing so all (i,j) pairs are covered exactly once.
  PE total: 32 eb + 3 rot + 32 rep + 128 main + 1 W1 = 196 passes.
- host does only layout transforms (shard/permute/pack/bf16 cast) + out
  transpose + bias add.
"""
import sys
import numpy as np

for _p in ("/opt/trn_rl_repo", "/root/.axon_site/_ro/trn_rl_repo"):
    if _p not in sys.path:
        sys.path.insert(0, _p)

B, M, TWO, NN, D = 1024, 4, 2, 32, 128
OUT = 128
NCORES = 8
BC = B // NCORES          # batches per core
G = BC * M                # 512 rows per core
NIG = D // 4              # 32 slabs of 4 features (partition packing (q,n))
CH = 8                    # slabs per DMA chunk -> 4 chunks per side
NCHUNK = NIG // CH

_CACHE = {}


def _build_nc():
    import os
    import concourse.bacc as bacc_mod
    import concourse.mybir as mybir
    from concourse.tile import TileContext

    F32 = mybir.dt.float32
    BF16 = mybir.dt.bfloat16
    MUL = mybir.AluOpType.mult

    nc = bacc_mod.Bacc(None)

    d_naA = nc.declare_dram_parameter("naA", [128, NIG * G], BF16, isOutput=False)
    d_nbA = nc.declare_dram_parameter("nbA", [128, NIG * G], BF16, isOutput=False)
    d_maskA = nc.declare_dram_parameter("maskA", [128, G], BF16, isOutput=False)
    d_maskB = nc.declare_dram_parameter("maskB", [128, G], BF16, isOutput=False)
    d_selfT = nc.declare_dram_parameter("selfT", [D, G], BF16, isOutput=False)
    d_W1 = nc.declare_dram_parameter("W1a", [D, OUT], BF16, isOutput=False)
    d_W2 = nc.declare_dram_parameter("W2A", [D, NIG * 4 * OUT], BF16, isOutput=False)
    d_BIG = nc.declare_dram_parameter("BIG", [128, 252], BF16, isOutput=False)
    d_BD = nc.declare_dram_parameter("BD", [128, 128], BF16, isOutput=False)
    d_PROT = nc.declare_dram_parameter("PROT", [128, 3 * 128], BF16, isOutput=False)
    d_out = nc.declare_dram_parameter("outT", [OUT, G], F32, isOutput=True)

    NDVE = int(os.environ.get("PT_NDVE", "4"))
    LOOK = int(os.environ.get("REP_LOOK", "2"))
    REP_BUFS = int(os.environ.get("REP_BUFS", "4"))
    EBT_BUFS = int(os.environ.get("EBT_BUFS", "2"))

    with TileContext(nc) as tc:
        with (
            tc.tile_pool(name="const", bufs=1) as cpool,
            tc.tile_pool(name="big", bufs=1) as bigpool,
            tc.tile_pool(name="rq", bufs=3) as rqpool,
            tc.tile_pool(name="pt", bufs=3) as ptpool,
            tc.tile_pool(name="misc", bufs=1) as mpool,
            tc.tile_pool(name="ps_ebt", bufs=EBT_BUFS, space="PSUM") as ps_ebt,
            tc.tile_pool(name="ps_rep", bufs=REP_BUFS, space="PSUM") as ps_rep,
            tc.tile_pool(name="ps_out", bufs=1, space="PSUM") as ps_out,
        ):
            # small constants first: maskB/BIG/PROT gate the eb phase
            maskB_t = cpool.tile([128, G], BF16, tag="mb")
            nc.sync.dma_start(out=maskB_t[:], in_=d_maskB[:])
            big_t = cpool.tile([128, 252], BF16, tag="big")
            nc.sync.dma_start(out=big_t[:], in_=d_BIG[:])
            prot_t = cpool.tile([128, 3 * 128], BF16, tag="prot")
            nc.sync.dma_start(out=prot_t[:], in_=d_PROT[:])

            # full-resident buffers; nb/na land in-place, masks applied
            # in-place (read-before-write on the streaming DVE is safe)
            mb_full = bigpool.tile([128, NIG * G], BF16, tag="mbF")
            ma_full = bigpool.tile([128, NIG * G], BF16, tag="maF")
            w2_full = bigpool.tile([128, NIG * 4 * OUT], BF16, tag="w2F")
            ebTall = bigpool.tile([128, 4 * G], BF16, tag="ebAll")

            # heavy DMAs in priority order on the sync queue: nb first
            # (gates the serial eb chain), tapered chunks so the last slabs
            # land with minimal tail; then na0, the rest of the consts,
            # W2 chunk 0 (gates first main matmuls), then the rest.
            NB_CHUNKS = [(0, 8), (8, 8), (16, 8), (24, 4), (28, 2), (30, 1), (31, 1)]
            for s0, sl in NB_CHUNKS:
                nc.sync.dma_start(
                    out=mb_full[:, s0 * G : (s0 + sl) * G],
                    in_=d_nbA[:, s0 * G : (s0 + sl) * G],
                )
            nc.sync.dma_start(
                out=ma_full[:, 0 : CH * G], in_=d_naA[:, 0 : CH * G]
            )
            maskA_t = cpool.tile([128, G], BF16, tag="ma")
            nc.sync.dma_start(out=maskA_t[:], in_=d_maskA[:])
            bd_t = cpool.tile([128, 128], BF16, tag="bd")
            nc.sync.dma_start(out=bd_t[:], in_=d_BD[:])
            selfT_t = cpool.tile([D, G], BF16, tag="sT")
            nc.sync.dma_start(out=selfT_t[:], in_=d_selfT[:])
            w1_t = cpool.tile([D, OUT], BF16, tag="w1")
            nc.sync.dma_start(out=w1_t[:], in_=d_W1[:])
            nc.sync.dma_start(
                out=ma_full[:, CH * G : 2 * CH * G],
                in_=d_naA[:, CH * G : 2 * CH * G],
            )
            nc.sync.dma_start(
                out=w2_full[:, 0 : CH * 4 * OUT], in_=d_W2[:, 0 : CH * 4 * OUT]
            )
            for c in range(2, NCHUNK):
                nc.sync.dma_start(
                    out=ma_full[:, c * CH * G : (c + 1) * CH * G],
                    in_=d_naA[:, c * CH * G : (c + 1) * CH * G],
                )
            for c in range(1, NCHUNK):
                nc.sync.dma_start(
                    out=w2_full[:, c * CH * 4 * OUT : (c + 1) * CH * 4 * OUT],
                    in_=d_W2[:, c * CH * 4 * OUT : (c + 1) * CH * 4 * OUT],
                )

            def emit_mask(full_t, mask_t, s0, sl):
                nc.vector.tensor_tensor(
                    out=full_t[:, s0 * G : (s0 + sl) * G].rearrange(
                        "p (s c) -> p s c", s=sl
                    ),
                    in0=full_t[:, s0 * G : (s0 + sl) * G].rearrange(
                        "p (s c) -> p s c", s=sl
                    ),
                    in1=mask_t[:][:, None, :].broadcast_to([128, sl, G]),
                    op=MUL,
                )

            # ---- EB phase: ebT[j, g] in PSUM via banded-selector matmuls ----
            p_ebt = ps_ebt.tile([128, G], F32, tag="ebt")
            for s0, sl in NB_CHUNKS:
                emit_mask(mb_full, maskB_t, s0, sl)
                for u in range(sl):
                    jg = s0 + u
                    nc.tensor.matmul(
                        p_ebt[:],
                        big_t[:, 124 - 4 * jg : 252 - 4 * jg],
                        mb_full[:, jg * G : (jg + 1) * G],
                        start=(jg == 0),
                        stop=(jg == NIG - 1),
                    )

            # ebT -> SBUF bf16 (rotation 0); rotations 1-3 via PE permutation
            # matmuls (PE is otherwise idle here); copies split Act/DVE.
            nc.scalar.copy(out=ebTall[:, 0:G], in_=p_ebt[:])
            rot_ps = []
            for r in range(1, 4):
                p_rot = ps_ebt.tile([128, G], F32, tag="ebt")
                nc.tensor.matmul(
                    p_rot[:],
                    prot_t[:, (r - 1) * 128 : r * 128],
                    ebTall[:, 0:G],
                    start=True,
                    stop=True,
                )
                if r == 2:
                    nc.vector.tensor_scalar_mul(
                        ebTall[:, r * G : (r + 1) * G], p_rot[:], 1.0
                    )
                else:
                    nc.scalar.copy(
                        out=ebTall[:, r * G : (r + 1) * G], in_=p_rot[:]
                    )
            emit_mask(ma_full, maskA_t, 0, CH)  # ma0 for first reps

            # ---- MAIN phase ----
            p_out = ps_out.tile([OUT, G], F32, tag="out")
            nc.tensor.matmul(p_out[:], w1_t[:], selfT_t[:], start=True, stop=False)

            rep_tiles = {}
            rq_tiles = {}
            pth_tiles = {}

            def emit_rep(ig):
                rep = ps_rep.tile([128, G], F32, tag="rep")
                nc.tensor.matmul(
                    rep[:],
                    bd_t[:],
                    ma_full[:, ig * G : (ig + 1) * G],
                    start=True,
                    stop=True,
                )
                rep_tiles[ig] = rep

            def emit_cp(ig):
                rep = rep_tiles.pop(ig)
                rq = rqpool.tile([128, G], BF16, tag="rq")
                nc.scalar.copy(out=rq[:], in_=rep[:])
                rq_tiles[ig] = rq

            def emit_pth(ig):
                rq = rq_tiles.pop(ig)
                pth = ptpool.tile([128, 4 * G], BF16, tag="pth")
                if NDVE > 0:
                    nc.vector.tensor_tensor(
                        out=pth[:, 0 : NDVE * G].rearrange("p (r c) -> p r c", r=NDVE),
                        in0=ebTall[:, 0 : NDVE * G].rearrange("p (r c) -> p r c", r=NDVE),
                        in1=rq[:][:, None, :].broadcast_to([128, NDVE, G]),
                        op=MUL,
                    )
                if NDVE < 4:
                    nrest = 4 - NDVE
                    nc.gpsimd.tensor_tensor(
                        out=pth[:, NDVE * G :].rearrange("p (r c) -> p r c", r=nrest),
                        in0=ebTall[:, NDVE * G :].rearrange("p (r c) -> p r c", r=nrest),
                        in1=rq[:][:, None, :].broadcast_to([128, nrest, G]),
                        op=MUL,
                    )
                pth_tiles[ig] = pth

            for ig in range(min(LOOK + 1, NIG)):
                emit_rep(ig)
            emit_cp(0)
            emit_pth(0)
            for ig in range(NIG):
                if ig == 0:
                    emit_mask(ma_full, maskA_t, CH, CH)  # late na masks in main loop
                if ig == 2:
                    emit_mask(ma_full, maskA_t, 2 * CH, CH)
                if ig == 6:
                    emit_mask(ma_full, maskA_t, 3 * CH, CH)
                if ig + LOOK + 1 < NIG:
                    emit_rep(ig + LOOK + 1)
                if ig + 1 < NIG:
                    emit_cp(ig + 1)
                    emit_pth(ig + 1)
                pth = pth_tiles.pop(ig)
                for r in range(4):
                    nc.tensor.matmul(
                        p_out[:],
                        w2_full[:, (ig * 4 + r) * OUT : (ig * 4 + r + 1) * OUT],
                        pth[:, r * G : (r + 1) * G],
                        start=False,
                        stop=(ig == NIG - 1 and r == 3),
                    )

            out_sb = mpool.tile([OUT, G], F32, tag="osb")
            nc.scalar.copy(out=out_sb[:, 0 : G // 2], in_=p_out[:, 0 : G // 2])
            nc.sync.dma_start(out=d_out[:, 0 : G // 2], in_=out_sb[:, 0 : G // 2])
            nc.scalar.copy(out=out_sb[:, G // 2 :], in_=p_out[:, G // 2 :])
            nc.sync.dma_start(out=d_out[:, G // 2 :], in_=out_sb[:, G // 2 :])

    nc.finalize()
    return nc


def _host_prep(self_vectors, neighbor_vectors, masks, W):
    import ml_dtypes

    f32 = np.float32
    bf16 = ml_dtypes.bfloat16
    sv = np.asarray(self_vectors, dtype=f32)
    nv = np.asarray(neighbor_vectors, dtype=f32)
    mk = np.asarray(masks, dtype=f32)
    Wf = np.asarray(W, dtype=f32)

    # per-core packs: partition p = (q, n) holds feature j = 4*ig + q
    # cols = (ig, g)
    nvc = nv.reshape(NCORES, G, TWO, NN, D)          # [c, g, t, n, d]

    def pack_side(t):
        arr = nvc[:, :, t]                            # [c, g, n, d]
        arr = arr.transpose(0, 3, 2, 1)               # [c, d, n, g]
        arr = arr.reshape(NCORES, NIG, 4, NN, G)      # [c, ig, q, n, g]
        arr = arr.transpose(0, 2, 3, 1, 4)            # [c, q, n, ig, g]
        return np.ascontiguousarray(
            arr.reshape(NCORES, 128, NIG * G).astype(bf16)
        )

    naA = pack_side(0)
    nbA = pack_side(1)

    mkc = mk.reshape(NCORES, G, TWO, NN)             # [c, g, t, n]
    mA = mkc[:, :, 0].transpose(0, 2, 1)             # [c, n, g]
    mB = mkc[:, :, 1].transpose(0, 2, 1)
    maskA = np.ascontiguousarray(
        np.broadcast_to(mA[:, None], (NCORES, 4, NN, G)).reshape(NCORES, 128, G).astype(bf16)
    )
    maskB = np.ascontiguousarray(
        np.broadcast_to(mB[:, None], (NCORES, 4, NN, G)).reshape(NCORES, 128, G).astype(bf16)
    )
    selfT = np.ascontiguousarray(
        sv.reshape(NCORES, G, D).transpose(0, 2, 1).astype(bf16)
    )  # [c, d, g]

    # shared weights
    W1a = np.ascontiguousarray(Wf[:, :D].T.astype(bf16))          # [d, o]
    w2 = Wf[:, D:].reshape(OUT, D, D)                             # [o, i, j]
    # W2A[p, (ig, r, o)] = w2[o, 4*ig + p//32, (p + 32*r) % 128]
    w2t = np.ascontiguousarray(w2.transpose(1, 2, 0))             # [i, j, o]
    p = np.arange(128)
    q = p // 32
    ig = np.arange(NIG)
    r = np.arange(4)
    i_full = 4 * ig[None, :] + q[:, None]                         # [p, ig]
    j_idx = (p[:, None] + 32 * r[None, :]) % 128                  # [p, r]
    W2A = w2t[i_full[:, :, None], j_idx[:, None, :]]              # [p, ig, r, o]
    W2A = np.ascontiguousarray(W2A.reshape(128, NIG * 4 * OUT).astype(bf16))

    BIG = np.zeros((128, 252), f32)
    rr = np.arange(128)
    BIG[rr, 124 + rr // 32] = 1.0 / 32.0
    BIG = BIG.astype(bf16)
    BD = np.zeros((128, 128), f32)
    BD[rr[:, None] // 32 == rr[None, :] // 32] = 1.0 / 32.0
    BD = BD.astype(bf16)
    # PROT[c, (r-1)*128 + p] = 1 iff c == (p + 32*r) % 128  (rotation matmuls)
    PROT = np.zeros((128, 3 * 128), f32)
    for r_ in range(1, 4):
        pp = np.arange(128)
        PROT[(pp + 32 * r_) % 128, (r_ - 1) * 128 + pp] = 1.0
    PROT = PROT.astype(bf16)

    in_maps = []
    for c in range(NCORES):
        in_maps.append(
            {
                "naA": naA[c],
                "nbA": nbA[c],
                "maskA": maskA[c],
                "maskB": maskB[c],
                "selfT": selfT[c],
                "W1a": W1a,
                "W2A": W2A,
                "BIG": BIG,
                "BD": BD,
                "PROT": PROT,
            }
        )
    return in_maps


def kernel(self_vectors, neighbor_vectors, masks, W, b):
    from concourse.bass_utils import run_bass_kernel_spmd

    if "nc" not in _CACHE:
        _CACHE["nc"] = _build_nc()
    nc = _CACHE["nc"]
    in_maps = _host_prep(self_vectors, neighbor_vectors, masks, W)
    results = run_bass_kernel_spmd(nc, in_maps, list(range(NCORES))).results
    out = np.empty((B, M, OUT), np.float32)
    for c in range(NCORES):
        out[c * BC : (c + 1) * BC] = (
            results[c]["outT"].T.reshape(BC, M, OUT)
        )
    out += np.asarray(b, np.float32)[None, None, :]
    return out


# revision 22
# speedup vs baseline: 538.3158x; 1.0118x over previous
"""Trainium2 Bass kernel for nn_CrossAggregator (gnn_message_passing).

out[g,o] = self[g]·W1[o,:] + ea_g^T A_o eb_g,  g=(b,m), A_o = W[o,128:].reshape(128,128)
ea/eb = masked means over 32 neighbors (t=0 / t=1).

Design v3 (per core, batch/8 data-parallel, G=512 rows), all heavy data bf16:
- single DMA queue (sync), strict priority order: consts, nb, na, W2 —
  chunks land in-place in full-resident tiles (no pool-recycle stalls).
- eb-side: masked-mean via 32 bf16 matmuls with a banded selector (BIG) as
  stationary -> ebT [j,g] in PSUM; Act-copied to SBUF bf16; 3 partition
  rotations ebT[(p+32r)%128, g] via PE permutation matmuls + Act copies
  -> ebTall [128, 4G].
- ea-side: ONE matmul per slab with a block-diagonal ones/32 stationary (BD)
  broadcasts all 4 i's of the slab into 4 partition bands -> repQ [128,G]
  (32 rep passes instead of 128).
- pt: repQ PSUM -> Act copy to bf16 -> one DVE 2x multiply per slab against
  the 4 ebT rotations (in0 dense, in1 stride-0 broadcast) -> pth [128, 4G].
- main contraction: 4 matmuls per slab, stationary W2stat[ig,r][p,o] =
  W2[o, 4ig+p//32, (p+32r)%128]; the rotation is folded into host W2
  packing so all (i,j) pairs are covered exactly once.
  PE total: 32 eb + 3 rot + 32 rep + 128 main + 1 W1 = 196 passes.
- host does only layout transforms (shard/permute/pack/bf16 cast) + out
  transpose + bias add.
"""
import sys
import numpy as np

for _p in ("/opt/trn_rl_repo", "/root/.axon_site/_ro/trn_rl_repo"):
    if _p not in sys.path:
        sys.path.insert(0, _p)

B, M, TWO, NN, D = 1024, 4, 2, 32, 128
OUT = 128
NCORES = 8
BC = B // NCORES          # batches per core
G = BC * M                # 512 rows per core
NIG = D // 4              # 32 slabs of 4 features (partition packing (q,n))
CH = 8                    # slabs per DMA chunk -> 4 chunks per side
NCHUNK = NIG // CH

_CACHE = {}


def _build_nc():
    import os
    import concourse.bacc as bacc_mod
    import concourse.mybir as mybir
    from concourse.tile import TileContext

    F32 = mybir.dt.float32
    BF16 = mybir.dt.bfloat16
    MUL = mybir.AluOpType.mult

    nc = bacc_mod.Bacc(None)

    d_naA = nc.declare_dram_parameter("naA", [128, NIG * G], BF16, isOutput=False)
    d_nbA = nc.declare_dram_parameter("nbA", [128, NIG * G], BF16, isOutput=False)
    d_maskA = nc.declare_dram_parameter("maskA", [128, G], BF16, isOutput=False)
    d_maskB = nc.declare_dram_parameter("maskB", [128, G], BF16, isOutput=False)
    d_selfT = nc.declare_dram_parameter("selfT", [D, G], BF16, isOutput=False)
    d_W1 = nc.declare_dram_parameter("W1a", [D, OUT], BF16, isOutput=False)
    d_W2 = nc.declare_dram_parameter("W2A", [D, NIG * 4 * OUT], BF16, isOutput=False)
    d_BIG = nc.declare_dram_parameter("BIG", [128, 252], BF16, isOutput=False)
    d_BD = nc.declare_dram_parameter("BD", [128, 128], BF16, isOutput=False)
    d_PROT = nc.declare_dram_parameter("PROT", [128, 3 * 128], BF16, isOutput=False)
    d_out = nc.declare_dram_parameter("outT", [OUT, G], F32, isOutput=True)

    NDVE = int(os.environ.get("PT_NDVE", "4"))
    LOOK = int(os.environ.get("REP_LOOK", "2"))
    REP_BUFS = int(os.environ.get("REP_BUFS", "4"))
    EBT_BUFS = int(os.environ.get("EBT_BUFS", "2"))

    with TileContext(nc) as tc:
        with (
            tc.tile_pool(name="const", bufs=1) as cpool,
            tc.tile_pool(name="big", bufs=1) as bigpool,
            tc.tile_pool(name="rq", bufs=3) as rqpool,
            tc.tile_pool(name="pt", bufs=3) as ptpool,
            tc.tile_pool(name="misc", bufs=1) as mpool,
            tc.tile_pool(name="ps_ebt", bufs=EBT_BUFS, space="PSUM") as ps_ebt,
            tc.tile_pool(name="ps_rep", bufs=REP_BUFS, space="PSUM") as ps_rep,
            tc.tile_pool(name="ps_out", bufs=1, space="PSUM") as ps_out,
        ):
            # maskB gates the very first mb mask multiply; everything else
            # can trail the first nb chunk (each DMA issue costs ~0.6us on
            # the sync engine, so nb0 goes as early as possible).
            maskB_t = cpool.tile([128, G], BF16, tag="mb")
            nc.sync.dma_start(out=maskB_t[:], in_=d_maskB[:])

            # full-resident buffers; nb/na land in-place, masks applied
            # in-place (read-before-write on the streaming DVE is safe)
            mb_full = bigpool.tile([128, NIG * G], BF16, tag="mbF")
            ma_full = bigpool.tile([128, NIG * G], BF16, tag="maF")
            w2_full = bigpool.tile([128, NIG * 4 * OUT], BF16, tag="w2F")
            ebTall = bigpool.tile([128, 4 * G], BF16, tag="ebAll")

            # heavy DMAs in priority order on the sync queue: nb first
            # (gates the serial eb chain), tapered chunks so the last slabs
            # land with minimal tail; then na0, the rest of the consts,
            # W2 chunk 0 (gates first main matmuls), then the rest.
            NB_CHUNKS = [(0, 8), (8, 8), (16, 8), (24, 4), (28, 2), (30, 1), (31, 1)]
            big_t = cpool.tile([128, 252], BF16, tag="big")
            prot_t = cpool.tile([128, 3 * 128], BF16, tag="prot")
            for ci, (s0, sl) in enumerate(NB_CHUNKS):
                nc.sync.dma_start(
                    out=mb_full[:, s0 * G : (s0 + sl) * G],
                    in_=d_nbA[:, s0 * G : (s0 + sl) * G],
                )
                if ci == 0:
                    # BIG gates the first eb matmul (~2.5us after mb0 lands)
                    nc.sync.dma_start(out=big_t[:], in_=d_BIG[:])
                elif ci == 1:
                    # PROT gates the rotation matmuls (~10us later)
                    nc.sync.dma_start(out=prot_t[:], in_=d_PROT[:])

            def na_dma(s0, sl):
                nc.sync.dma_start(
                    out=ma_full[:, s0 * G : (s0 + sl) * G],
                    in_=d_naA[:, s0 * G : (s0 + sl) * G],
                )

            def w2_dma(ig0, nig):
                nc.sync.dma_start(
                    out=w2_full[:, ig0 * 4 * OUT : (ig0 + nig) * 4 * OUT],
                    in_=d_W2[:, ig0 * 4 * OUT : (ig0 + nig) * 4 * OUT],
                )

            # rep0 chain (na[0:4] -> ma -> rep -> rq -> pth0 -> main0) and
            # the first main matmuls (W2 igs 0-1) right behind the nb stream;
            # then na / W2 interleaved against their just-in-time deadlines.
            na_dma(0, 4)
            w2_dma(0, 2)
            maskA_t = cpool.tile([128, G], BF16, tag="ma")
            nc.sync.dma_start(out=maskA_t[:], in_=d_maskA[:])
            bd_t = cpool.tile([128, 128], BF16, tag="bd")
            nc.sync.dma_start(out=bd_t[:], in_=d_BD[:])
            selfT_t = cpool.tile([D, G], BF16, tag="sT")
            nc.sync.dma_start(out=selfT_t[:], in_=d_selfT[:])
            w1_t = cpool.tile([D, OUT], BF16, tag="w1")
            nc.sync.dma_start(out=w1_t[:], in_=d_W1[:])
            na_dma(4, 4)
            w2_dma(2, 6)
            na_dma(8, 8)
            w2_dma(8, 8)
            na_dma(16, 8)
            w2_dma(16, 8)
            na_dma(24, 8)
            w2_dma(24, 8)

            def emit_mask(full_t, mask_t, s0, sl, eng=None):
                (eng or nc.vector).tensor_tensor(
                    out=full_t[:, s0 * G : (s0 + sl) * G].rearrange(
                        "p (s c) -> p s c", s=sl
                    ),
                    in0=full_t[:, s0 * G : (s0 + sl) * G].rearrange(
                        "p (s c) -> p s c", s=sl
                    ),
                    in1=mask_t[:][:, None, :].broadcast_to([128, sl, G]),
                    op=MUL,
                )

            # ---- EB phase: ebT[j, g] in PSUM via banded-selector matmuls ----
            p_ebt = ps_ebt.tile([128, G], F32, tag="ebt")
            for s0, sl in NB_CHUNKS:
                emit_mask(mb_full, maskB_t, s0, sl)
                for u in range(sl):
                    jg = s0 + u
                    nc.tensor.matmul(
                        p_ebt[:],
                        big_t[:, 124 - 4 * jg : 252 - 4 * jg],
                        mb_full[:, jg * G : (jg + 1) * G],
                        start=(jg == 0),
                        stop=(jg == NIG - 1),
                    )

            # ebT -> SBUF bf16 (rotation 0); rotations 1-3 via PE permutation
            # matmuls (PE is otherwise idle here); copies split Act/DVE.
            nc.scalar.copy(out=ebTall[:, 0:G], in_=p_ebt[:])
            rot_ps = []
            for r in range(1, 4):
                p_rot = ps_ebt.tile([128, G], F32, tag="ebt")
                nc.tensor.matmul(
                    p_rot[:],
                    prot_t[:, (r - 1) * 128 : r * 128],
                    ebTall[:, 0:G],
                    start=True,
                    stop=True,
                )
                nc.scalar.copy(
                    out=ebTall[:, r * G : (r + 1) * G], in_=p_rot[:]
                )
            emit_mask(ma_full, maskA_t, 0, 4)  # slabs 0-3 gate reps 0-3

            # ---- MAIN phase ----
            p_out = ps_out.tile([OUT, G], F32, tag="out")
            nc.tensor.matmul(p_out[:], w1_t[:], selfT_t[:], start=True, stop=False)

            rep_tiles = {}
            rq_tiles = {}
            pth_tiles = {}

            def emit_rep(ig):
                rep = ps_rep.tile([128, G], F32, tag="rep")
                nc.tensor.matmul(
                    rep[:],
                    bd_t[:],
                    ma_full[:, ig * G : (ig + 1) * G],
                    start=True,
                    stop=True,
                )
                rep_tiles[ig] = rep

            def emit_cp(ig):
                rep = rep_tiles.pop(ig)
                rq = rqpool.tile([128, G], BF16, tag="rq")
                nc.scalar.copy(out=rq[:], in_=rep[:])
                rq_tiles[ig] = rq

            def emit_pth(ig):
                rq = rq_tiles.pop(ig)
                pth = ptpool.tile([128, 4 * G], BF16, tag="pth")
                if NDVE > 0:
                    nc.vector.tensor_tensor(
                        out=pth[:, 0 : NDVE * G].rearrange("p (r c) -> p r c", r=NDVE),
                        in0=ebTall[:, 0 : NDVE * G].rearrange("p (r c) -> p r c", r=NDVE),
                        in1=rq[:][:, None, :].broadcast_to([128, NDVE, G]),
                        op=MUL,
                    )
                if NDVE < 4:
                    nrest = 4 - NDVE
                    nc.gpsimd.tensor_tensor(
                        out=pth[:, NDVE * G :].rearrange("p (r c) -> p r c", r=nrest),
                        in0=ebTall[:, NDVE * G :].rearrange("p (r c) -> p r c", r=nrest),
                        in1=rq[:][:, None, :].broadcast_to([128, nrest, G]),
                        op=MUL,
                    )
                pth_tiles[ig] = pth

            for ig in range(min(LOOK + 1, NIG)):
                emit_rep(ig)
            emit_cp(0)
            emit_pth(0)
            # remaining na masks threaded through the main loop in 4-slab
            # pieces: rep(ig+3) emitted at loop ig needs slab ig+3 masked.
            # remaining na masks: alternate GpSimd (slow but otherwise idle)
            # and DVE (the pacer) — each piece ahead of its rep deadline.
            MA_SCHED = {0: (4, "p"), 1: (8, "v"), 3: (12, "p"), 5: (16, "v"),
                        7: (20, "p"), 9: (24, "v"), 11: (28, "p")}
            use_pool = os.environ.get("MA_POOL", "0") == "1"
            for ig in range(NIG):
                if ig in MA_SCHED:
                    s0, e = MA_SCHED[ig]
                    eng = nc.gpsimd if (use_pool and e == "p") else nc.vector
                    emit_mask(ma_full, maskA_t, s0, 4, eng=eng)
                if ig + LOOK + 1 < NIG:
                    emit_rep(ig + LOOK + 1)
                if ig + 1 < NIG:
                    emit_cp(ig + 1)
                    emit_pth(ig + 1)
                pth = pth_tiles.pop(ig)
                for r in range(4):
                    nc.tensor.matmul(
                        p_out[:],
                        w2_full[:, (ig * 4 + r) * OUT : (ig * 4 + r + 1) * OUT],
                        pth[:, r * G : (r + 1) * G],
                        start=False,
                        stop=(ig == NIG - 1 and r == 3),
                    )

            out_sb = mpool.tile([OUT, G], F32, tag="osb")
            nc.scalar.copy(out=out_sb[:, 0 : G // 2], in_=p_out[:, 0 : G // 2])
            nc.sync.dma_start(out=d_out[:, 0 : G // 2], in_=out_sb[:, 0 : G // 2])
            nc.scalar.copy(out=out_sb[:, G // 2 :], in_=p_out[:, G // 2 :])
            nc.sync.dma_start(out=d_out[:, G // 2 :], in_=out_sb[:, G // 2 :])

    nc.finalize()
    return nc


def _host_prep(self_vectors, neighbor_vectors, masks, W):
    import ml_dtypes

    f32 = np.float32
    bf16 = ml_dtypes.bfloat16
    sv = np.asarray(self_vectors, dtype=f32)
    nv = np.asarray(neighbor_vectors, dtype=f32)
    mk = np.asarray(masks, dtype=f32)
    Wf = np.asarray(W, dtype=f32)

    # per-core packs: partition p = (q, n) holds feature j = 4*ig + q
    # cols = (ig, g)
    nvc = nv.reshape(NCORES, G, TWO, NN, D)          # [c, g, t, n, d]

    def pack_side(t):
        arr = nvc[:, :, t]                            # [c, g, n, d]
        arr = arr.transpose(0, 3, 2, 1)               # [c, d, n, g]
        arr = arr.reshape(NCORES, NIG, 4, NN, G)      # [c, ig, q, n, g]
        arr = arr.transpose(0, 2, 3, 1, 4)            # [c, q, n, ig, g]
        return np.ascontiguousarray(
            arr.reshape(NCORES, 128, NIG * G).astype(bf16)
        )

    naA = pack_side(0)
    nbA = pack_side(1)

    mkc = mk.reshape(NCORES, G, TWO, NN)             # [c, g, t, n]
    mA = mkc[:, :, 0].transpose(0, 2, 1)             # [c, n, g]
    mB = mkc[:, :, 1].transpose(0, 2, 1)
    maskA = np.ascontiguousarray(
        np.broadcast_to(mA[:, None], (NCORES, 4, NN, G)).reshape(NCORES, 128, G).astype(bf16)
    )
    maskB = np.ascontiguousarray(
        np.broadcast_to(mB[:, None], (NCORES, 4, NN, G)).reshape(NCORES, 128, G).astype(bf16)
    )
    selfT = np.ascontiguousarray(
        sv.reshape(NCORES, G, D).transpose(0, 2, 1).astype(bf16)
    )  # [c, d, g]

    # shared weights
    W1a = np.ascontiguousarray(Wf[:, :D].T.astype(bf16))          # [d, o]
    w2 = Wf[:, D:].reshape(OUT, D, D)                             # [o, i, j]
    # W2A[p, (ig, r, o)] = w2[o, 4*ig + p//32, (p + 32*r) % 128]
    w2t = np.ascontiguousarray(w2.transpose(1, 2, 0))             # [i, j, o]
    p = np.arange(128)
    q = p // 32
    ig = np.arange(NIG)
    r = np.arange(4)
    i_full = 4 * ig[None, :] + q[:, None]                         # [p, ig]
    j_idx = (p[:, None] + 32 * r[None, :]) % 128                  # [p, r]
    W2A = w2t[i_full[:, :, None], j_idx[:, None, :]]              # [p, ig, r, o]
    W2A = np.ascontiguousarray(W2A.reshape(128, NIG * 4 * OUT).astype(bf16))

    BIG = np.zeros((128, 252), f32)
    rr = np.arange(128)
    BIG[rr, 124 + rr // 32] = 1.0 / 32.0
    BIG = BIG.astype(bf16)
    BD = np.zeros((128, 128), f32)
    BD[rr[:, None] // 32 == rr[None, :] // 32] = 1.0 / 32.0
    BD = BD.astype(bf16)
    # PROT[c, (r-1)*128 + p] = 1 iff c == (p + 32*r) % 128  (rotation matmuls)
    PROT = np.zeros((128, 3 * 128), f32)
    for r_ in range(1, 4):
        pp = np.arange(128)
        PROT[(pp + 32 * r_) % 128, (r_ - 1) * 128 + pp] = 1.0
    PROT = PROT.astype(bf16)

    in_maps = []
    for c in range(NCORES):
        in_maps.append(
            {
                "naA": naA[c],
                "nbA": nbA[c],
                "maskA": maskA[c],
                "maskB": maskB[c],
                "selfT": selfT[c],
                "W1a": W1a,
                "W2A": W2A,
                "BIG": BIG,
                "BD": BD,
                "PROT": PROT,
            }
        )
    return in_maps


def kernel(self_vectors, neighbor_vectors, masks, W, b):
    from concourse.bass_utils import run_bass_kernel_spmd

    if "nc" not in _CACHE:
        _CACHE["nc"] = _build_nc()
    nc = _CACHE["nc"]
    in_maps = _host_prep(self_vectors, neighbor_vectors, masks, W)
    results = run_bass_kernel_spmd(nc, in_maps, list(range(NCORES))).results
    out = np.empty((B, M, OUT), np.float32)
    for c in range(NCORES):
        out[c * BC : (c + 1) * BC] = (
            results[c]["outT"].T.reshape(BC, M, OUT)
        )
    out += np.asarray(b, np.float32)[None, None, :]
    return out


# revision 37
# speedup vs baseline: 540.9452x; 1.0049x over previous
"""Trainium2 Bass kernel for nn_CrossAggregator (gnn_message_passing).

out[g,o] = self[g]·W1[o,:] + ea_g^T A_o eb_g,  g=(b,m), A_o = W[o,128:].reshape(128,128)
ea/eb = masked means over 32 neighbors (t=0 / t=1).

Design v3 (per core, batch/8 data-parallel, G=512 rows), all heavy data bf16:
- single DMA queue (sync), strict priority order: consts, nb, na, W2 —
  chunks land in-place in full-resident tiles (no pool-recycle stalls).
- eb-side: masked-mean via 32 bf16 matmuls with a banded selector (BIG) as
  stationary -> ebT [j,g] in PSUM; Act-copied to SBUF bf16; 3 partition
  rotations ebT[(p+32r)%128, g] via PE permutation matmuls + Act copies
  -> ebTall [128, 4G].
- ea-side: ONE matmul per slab with a block-diagonal ones/32 stationary (BD)
  broadcasts all 4 i's of the slab into 4 partition bands -> repQ [128,G]
  (32 rep passes instead of 128).
- pt: repQ PSUM -> Act copy to bf16 -> one DVE 2x multiply per slab against
  the 4 ebT rotations (in0 dense, in1 stride-0 broadcast) -> pth [128, 4G].
- main contraction: 4 matmuls per slab, stationary W2stat[ig,r][p,o] =
  W2[o, 4ig+p//32, (p+32r)%128]; the rotation is folded into host W2
  packing so all (i,j) pairs are covered exactly once.
  PE total: 32 eb + 3 rot + 32 rep + 128 main + 1 W1 = 196 passes.
- host does only layout transforms (shard/permute/pack/bf16 cast) + out
  transpose + bias add.
"""
import sys
import numpy as np

for _p in ("/opt/trn_rl_repo", "/root/.axon_site/_ro/trn_rl_repo"):
    if _p not in sys.path:
        sys.path.insert(0, _p)

B, M, TWO, NN, D = 1024, 4, 2, 32, 128
OUT = 128
NCORES = 8
BC = B // NCORES          # batches per core
G = BC * M                # 512 rows per core
NIG = D // 4              # 32 slabs of 4 features (partition packing (q,n))
CH = 8                    # slabs per DMA chunk -> 4 chunks per side
NCHUNK = NIG // CH

_CACHE = {}


def _build_nc():
    import os
    import concourse.bacc as bacc_mod
    import concourse.mybir as mybir
    from concourse.tile import TileContext

    F32 = mybir.dt.float32
    BF16 = mybir.dt.bfloat16
    MUL = mybir.AluOpType.mult

    nc = bacc_mod.Bacc(None)

    d_naA = nc.declare_dram_parameter("naA", [128, NIG * G], BF16, isOutput=False)
    d_nbA = nc.declare_dram_parameter("nbA", [128, NIG * G], BF16, isOutput=False)
    d_maskA = nc.declare_dram_parameter("maskA", [128, G], BF16, isOutput=False)
    d_maskB = nc.declare_dram_parameter("maskB", [128, G], BF16, isOutput=False)
    d_selfT = nc.declare_dram_parameter("selfT", [D, G], BF16, isOutput=False)
    d_W1 = nc.declare_dram_parameter("W1a", [D, OUT], BF16, isOutput=False)
    d_W2 = nc.declare_dram_parameter("W2A", [D, NIG * 4 * OUT], BF16, isOutput=False)
    d_BIG = nc.declare_dram_parameter("BIG", [128, 252], BF16, isOutput=False)
    d_BD = nc.declare_dram_parameter("BD", [128, 128], BF16, isOutput=False)
    d_PROT = nc.declare_dram_parameter("PROT", [128, 3 * 128], BF16, isOutput=False)
    d_out = nc.declare_dram_parameter("outT", [OUT, G], F32, isOutput=True)

    NDVE = int(os.environ.get("PT_NDVE", "4"))
    LOOK = int(os.environ.get("REP_LOOK", "2"))
    REP_BUFS = int(os.environ.get("REP_BUFS", "4"))
    EBT_BUFS = int(os.environ.get("EBT_BUFS", "2"))

    with TileContext(nc) as tc:
        with (
            tc.tile_pool(name="const", bufs=1) as cpool,
            tc.tile_pool(name="big", bufs=1) as bigpool,
            tc.tile_pool(name="rq", bufs=3) as rqpool,
            tc.tile_pool(name="pt", bufs=3) as ptpool,
            tc.tile_pool(name="misc", bufs=1) as mpool,
            tc.tile_pool(name="ps_ebt", bufs=EBT_BUFS, space="PSUM") as ps_ebt,
            tc.tile_pool(name="ps_rep", bufs=REP_BUFS, space="PSUM") as ps_rep,
            tc.tile_pool(name="ps_out", bufs=1, space="PSUM") as ps_out,
        ):
            # maskB gates the very first mb mask multiply; everything else
            # can trail the first nb chunk (each DMA issue costs ~0.6us on
            # the sync engine, so nb0 goes as early as possible).
            maskB_t = cpool.tile([128, G], BF16, tag="mb")
            nc.sync.dma_start(out=maskB_t[:], in_=d_maskB[:])

            # full-resident buffers; nb/na land in-place, masks applied
            # in-place (read-before-write on the streaming DVE is safe)
            mb_full = bigpool.tile([128, NIG * G], BF16, tag="mbF")
            ma_full = bigpool.tile([128, NIG * G], BF16, tag="maF")
            w2_full = bigpool.tile([128, NIG * 4 * OUT], BF16, tag="w2F")
            # ebTall holds the 4 rotations twice ([0:4G] and [4G:8G]) so a
            # paired (2-slab) pth op can read a fully dense in0
            ebTall = bigpool.tile([128, 8 * G], BF16, tag="ebAll")

            # heavy DMAs in priority order on the sync queue: nb first
            # (gates the serial eb chain), tapered chunks so the last slabs
            # land with minimal tail; then na0, the rest of the consts,
            # W2 chunk 0 (gates first main matmuls), then the rest.
            NB_CHUNKS = [(0, 2), (2, 6), (8, 4), (12, 4), (16, 4), (20, 4),
                         (24, 4), (28, 2), (30, 1), (31, 1)]
            big_t = cpool.tile([128, 252], BF16, tag="big")
            prot_t = cpool.tile([128, 3 * 128], BF16, tag="prot")
            maskA_t = cpool.tile([128, G], BF16, tag="ma")
            bd_t = cpool.tile([128, 128], BF16, tag="bd")

            def na_dma(s0, sl):
                nc.sync.dma_start(
                    out=ma_full[:, s0 * G : (s0 + sl) * G],
                    in_=d_naA[:, s0 * G : (s0 + sl) * G],
                )

            for ci, (s0, sl) in enumerate(NB_CHUNKS):
                nc.sync.dma_start(
                    out=mb_full[:, s0 * G : (s0 + sl) * G],
                    in_=d_nbA[:, s0 * G : (s0 + sl) * G],
                )
                if ci == 0:
                    # BIG gates the first eb matmul (~1us after mb0 lands)
                    nc.sync.dma_start(out=big_t[:], in_=d_BIG[:])
                elif ci == 1:
                    # PROT gates the rotation matmuls (~12us later)
                    nc.sync.dma_start(out=prot_t[:], in_=d_PROT[:])
            def w2_dma(ig0, nig):
                nc.sync.dma_start(
                    out=w2_full[:, ig0 * 4 * OUT : (ig0 + nig) * 4 * OUT],
                    in_=d_W2[:, ig0 * 4 * OUT : (ig0 + nig) * 4 * OUT],
                )

            nc.sync.dma_start(out=maskA_t[:], in_=d_maskA[:])
            nc.sync.dma_start(out=bd_t[:], in_=d_BD[:])
            na_dma(0, 4)
            w2_dma(0, 2)
            na_dma(4, 4)
            selfT_t = cpool.tile([D, G], BF16, tag="sT")
            nc.sync.dma_start(out=selfT_t[:], in_=d_selfT[:])
            w1_t = cpool.tile([D, OUT], BF16, tag="w1")
            nc.sync.dma_start(out=w1_t[:], in_=d_W1[:])
            w2_dma(2, 6)
            na_dma(8, 8)
            w2_dma(8, 8)
            na_dma(16, 8)
            w2_dma(16, 8)
            na_dma(24, 8)
            w2_dma(24, 8)

            def emit_mask(full_t, mask_t, s0, sl, eng=None):
                (eng or nc.vector).tensor_tensor(
                    out=full_t[:, s0 * G : (s0 + sl) * G].rearrange(
                        "p (s c) -> p s c", s=sl
                    ),
                    in0=full_t[:, s0 * G : (s0 + sl) * G].rearrange(
                        "p (s c) -> p s c", s=sl
                    ),
                    in1=mask_t[:][:, None, :].broadcast_to([128, sl, G]),
                    op=MUL,
                )

            # ---- EB phase: ebT[j, g] in PSUM via banded-selector matmuls ----
            p_ebt = ps_ebt.tile([128, G], F32, tag="ebt")
            for ci, (s0, sl) in enumerate(NB_CHUNKS):
                emit_mask(mb_full, maskB_t, s0, sl)
                for u in range(sl):
                    jg = s0 + u
                    nc.tensor.matmul(
                        p_ebt[:],
                        big_t[:, 124 - 4 * jg : 252 - 4 * jg],
                        mb_full[:, jg * G : (jg + 1) * G],
                        start=(jg == 0),
                        stop=(jg == NIG - 1),
                    )


            # ma[0:4] right behind the mb-mask tail (na[0:4] lands just
            # after the nb stream); gates reps 0-3
            emit_mask(ma_full, maskA_t, 0, 4)

            # ebT -> SBUF bf16 (rotation 0); rotations 1-3 via PE permutation
            # matmuls (PE is otherwise idle here) + Act copies; then one DVE
            # 4x copy duplicates [0:4G] -> [4G:8G] for the paired pth in0.
            nc.scalar.copy(out=ebTall[:, 0:G], in_=p_ebt[:])
            for r in range(1, 4):
                p_rot = ps_ebt.tile([128, G], F32, tag="ebt")
                nc.tensor.matmul(
                    p_rot[:],
                    prot_t[:, (r - 1) * 128 : r * 128],
                    ebTall[:, 0:G],
                    start=True,
                    stop=True,
                )
                nc.scalar.copy(
                    out=ebTall[:, r * G : (r + 1) * G], in_=p_rot[:]
                )


            # ---- MAIN phase ----
            p_out = ps_out.tile([OUT, G], F32, tag="out")
            nc.tensor.matmul(p_out[:], w1_t[:], selfT_t[:], start=True, stop=False)

            rep_tiles = {}
            rq_tiles = {}
            pth_tiles = {}
            NPAIR = NIG // 2

            def emit_rep(ig):
                rep = ps_rep.tile([128, G], F32, tag="rep")
                nc.tensor.matmul(
                    rep[:],
                    bd_t[:],
                    ma_full[:, ig * G : (ig + 1) * G],
                    start=True,
                    stop=True,
                )
                rep_tiles[ig] = rep

            def emit_cp(k):
                # rq pair tile: halves written by two Act copies
                rq = rqpool.tile([128, 2 * G], BF16, tag="rq")
                for u in range(2):
                    rep = rep_tiles.pop(2 * k + u)
                    nc.scalar.copy(out=rq[:, u * G : (u + 1) * G], in_=rep[:])
                rq_tiles[k] = rq

            def emit_pth(k, split=False):
                # one DVE op per slab PAIR: out/in0 fully dense 4D, in1
                # broadcasts each slab's rq across the 4 rotations.
                # split=True: two 4G ops reading ebTall[0:4G] only — used
                # for pair 0 (before the dup lands) and the last pair
                # (first 4 main matmuls start half an op earlier).
                rq = rq_tiles.pop(k)
                pth = ptpool.tile([128, 8 * G], BF16, tag="pth")
                if split:
                    for u in range(2):
                        nc.vector.tensor_tensor(
                            out=pth[:, u * 4 * G : (u + 1) * 4 * G].rearrange(
                                "p (r c) -> p r c", r=4
                            ),
                            in0=ebTall[:, 0 : 4 * G].rearrange(
                                "p (r c) -> p r c", r=4
                            ),
                            in1=rq[:, u * G : (u + 1) * G][:, None, :]
                            .broadcast_to([128, 4, G]),
                            op=MUL,
                        )
                else:
                    nc.vector.tensor_tensor(
                        out=pth[:].rearrange("p (i r c) -> p i r c", i=2, r=4),
                        in0=ebTall[:].rearrange("p (i r c) -> p i r c", i=2, r=4),
                        in1=rq[:].rearrange("p (i c) -> p i c", i=2)[:, :, None, :]
                        .broadcast_to([128, 2, 4, G]),
                        op=MUL,
                    )
                pth_tiles[k] = pth

            for ig in range(2 * min(LOOK, NPAIR)):
                emit_rep(ig)
            emit_cp(0)
            emit_pth(0, split=True)
            # duplicate ebTall[0:4G] -> [4G:8G] for the paired in0 (4x DVE
            # copy, off the pth0 critical path)
            nc.vector.tensor_scalar_mul(
                ebTall[:, 4 * G : 8 * G], ebTall[:, 0 : 4 * G], 1.0
            )
            # remaining na masks threaded through the pair loop in 8/4-slab
            # pieces, each well ahead of its rep deadline (rep(2k+2*LOOK+1)
            # is emitted at pair k).
            MA_SCHED = {0: (4, 4), 1: (8, 8), 2: (16, 8), 4: (24, 8)}
            for k in range(NPAIR):
                if k in MA_SCHED:
                    s0, sl = MA_SCHED[k]
                    emit_mask(ma_full, maskA_t, s0, sl)
                if k + LOOK < NPAIR:
                    emit_rep(2 * k + 2 * LOOK)
                    emit_rep(2 * k + 2 * LOOK + 1)
                if k + 1 < NPAIR:
                    emit_cp(k + 1)
                    emit_pth(k + 1, split=(k + 1 == NPAIR - 1))
                pth = pth_tiles.pop(k)
                for u in range(2):
                    ig = 2 * k + u
                    for r in range(4):
                        nc.tensor.matmul(
                            p_out[:],
                            w2_full[:, (ig * 4 + r) * OUT : (ig * 4 + r + 1) * OUT],
                            pth[:, (u * 4 + r) * G : (u * 4 + r + 1) * G],
                            start=False,
                            stop=(ig == NIG - 1 and r == 3),
                        )

            out_sb = mpool.tile([OUT, G], F32, tag="osb")
            nc.scalar.copy(out=out_sb[:, 0 : G // 2], in_=p_out[:, 0 : G // 2])
            nc.sync.dma_start(out=d_out[:, 0 : G // 2], in_=out_sb[:, 0 : G // 2])
            nc.scalar.copy(out=out_sb[:, G // 2 :], in_=p_out[:, G // 2 :])
            nc.sync.dma_start(out=d_out[:, G // 2 :], in_=out_sb[:, G // 2 :])

    nc.finalize()
    return nc


def _host_prep(self_vectors, neighbor_vectors, masks, W):
    import ml_dtypes

    f32 = np.float32
    bf16 = ml_dtypes.bfloat16
    sv = np.asarray(self_vectors, dtype=f32)
    nv = np.asarray(neighbor_vectors, dtype=f32)
    mk = np.asarray(masks, dtype=f32)
    Wf = np.asarray(W, dtype=f32)

    # per-core packs: partition p = (q, n) holds feature j = 4*ig + q
    # cols = (ig, g)
    nvc = nv.reshape(NCORES, G, TWO, NN, D)          # [c, g, t, n, d]

    def pack_side(t):
        arr = nvc[:, :, t]                            # [c, g, n, d]
        arr = arr.transpose(0, 3, 2, 1)               # [c, d, n, g]
        arr = arr.reshape(NCORES, NIG, 4, NN, G)      # [c, ig, q, n, g]
        arr = arr.transpose(0, 2, 3, 1, 4)            # [c, q, n, ig, g]
        return np.ascontiguousarray(
            arr.reshape(NCORES, 128, NIG * G).astype(bf16)
        )

    naA = pack_side(0)
    nbA = pack_side(1)

    mkc = mk.reshape(NCORES, G, TWO, NN)             # [c, g, t, n]
    mA = mkc[:, :, 0].transpose(0, 2, 1)             # [c, n, g]
    mB = mkc[:, :, 1].transpose(0, 2, 1)
    maskA = np.ascontiguousarray(
        np.broadcast_to(mA[:, None], (NCORES, 4, NN, G)).reshape(NCORES, 128, G).astype(bf16)
    )
    maskB = np.ascontiguousarray(
        np.broadcast_to(mB[:, None], (NCORES, 4, NN, G)).reshape(NCORES, 128, G).astype(bf16)
    )
    selfT = np.ascontiguousarray(
        sv.reshape(NCORES, G, D).transpose(0, 2, 1).astype(bf16)
    )  # [c, d, g]

    # shared weights
    W1a = np.ascontiguousarray(Wf[:, :D].T.astype(bf16))          # [d, o]
    w2 = Wf[:, D:].reshape(OUT, D, D)                             # [o, i, j]
    # W2A[p, (ig, r, o)] = w2[o, 4*ig + p//32, (p + 32*r) % 128]
    w2t = np.ascontiguousarray(w2.transpose(1, 2, 0))             # [i, j, o]
    p = np.arange(128)
    q = p // 32
    ig = np.arange(NIG)
    r = np.arange(4)
    i_full = 4 * ig[None, :] + q[:, None]                         # [p, ig]
    j_idx = (p[:, None] + 32 * r[None, :]) % 128                  # [p, r]
    W2A = w2t[i_full[:, :, None], j_idx[:, None, :]]              # [p, ig, r, o]
    W2A = np.ascontiguousarray(W2A.reshape(128, NIG * 4 * OUT).astype(bf16))

    BIG = np.zeros((128, 252), f32)
    rr = np.arange(128)
    BIG[rr, 124 + rr // 32] = 1.0 / 32.0
    BIG = BIG.astype(bf16)
    BD = np.zeros((128, 128), f32)
    BD[rr[:, None] // 32 == rr[None, :] // 32] = 1.0 / 32.0
    BD = BD.astype(bf16)
    # PROT[c, (r-1)*128 + p] = 1 iff c == (p + 32*r) % 128  (rotation matmuls)
    PROT = np.zeros((128, 3 * 128), f32)
    for r_ in range(1, 4):
        pp = np.arange(128)
        PROT[(pp + 32 * r_) % 128, (r_ - 1) * 128 + pp] = 1.0
    PROT = PROT.astype(bf16)

    in_maps = []
    for c in range(NCORES):
        in_maps.append(
            {
                "naA": naA[c],
                "nbA": nbA[c],
                "maskA": maskA[c],
                "maskB": maskB[c],
                "selfT": selfT[c],
                "W1a": W1a,
                "W2A": W2A,
                "BIG": BIG,
                "BD": BD,
                "PROT": PROT,
            }
        )
    return in_maps


def kernel(self_vectors, neighbor_vectors, masks, W, b):
    from concourse.bass_utils import run_bass_kernel_spmd

    if "nc" not in _CACHE:
        _CACHE["nc"] = _build_nc()
    nc = _CACHE["nc"]
    in_maps = _host_prep(self_vectors, neighbor_vectors, masks, W)
    results = run_bass_kernel_spmd(nc, in_maps, list(range(NCORES))).results
    out = np.empty((B, M, OUT), np.float32)
    for c in range(NCORES):
        out[c * BC : (c + 1) * BC] = (
            results[c]["outT"].T.reshape(BC, M, OUT)
        )
    out += np.asarray(b, np.float32)[None, None, :]
    return out
